# revision 5
# baseline (speedup 1.0000x reference)
"""Trainium2 Bass kernel for nn_AdaptiveMask: out = x * ring_mask(current_val).

x: [32, 8, 256, 256] f32.  mask: [256, 256] computed from the scalar
current_val (concentric-ring ramp, values in [0, 1]).

Strategy (memory-bound, pure elementwise):
  - Shard x along batch dim: 4 batches per core across 8 cores (data parallel).
  - Host precomputes the [256, 256] mask from current_val, then lays it out as
    a [128, 4096] "mega mask" that matches the SBUF tile layout of a contiguous
    2 MiB chunk of x, so the device does a plain tensor_tensor multiply with no
    broadcast logic.
  - Per core: 4 tiles of [128, 4096] f32 (2 MiB each, contiguous in HBM).
    Loads on nc.sync (HWDGE ring 0), multiply on DVE in-place, stores on
    nc.scalar (HWDGE ring 1). Tile framework handles pipelining (bufs=4).

Layout math: per-core shard [4, 8, 256, 256] viewed as [512, 4096] row-major.
Element (R, j) of that view is image row r = (R % 16) * 16 + j // 256 and
col c = j % 256 (every R spans 16 consecutive image rows; R % 16 is the
16-row group within one 256-row image). A [128, 4096] tile starting at
R = 128 t keeps the same mapping for every t because 128 ≡ 0 (mod 16)...
in fact each tile holds 8 complete [256, 256] images worth of rows, so one
mega-mask M[p, j] = mask[(p % 16) * 16 + j // 256, j % 256] serves all tiles.
"""

import sys

import numpy as np

for _p in ("/opt/trn_rl_repo",):
    if _p not in sys.path:
        sys.path.append(_p)

from concourse import bacc, bass, tile
from concourse.bass import mybir
from concourse.bass_utils import run_bass_kernel_spmd

N_CORES = 8
B, H, N = 32, 8, 256
MAX_SIZE = 256
RAMP_SIZE = 32

ROWS = (B // N_CORES) * H * N * N // 4096  # 512 rows of 4096 f32 per core
TILE_F = 4096
N_TILES = ROWS // 128  # 4

_cache = {}


def _build_program():
    nc = bacc.Bacc(None, target_bir_lowering=False)
    x_in = nc.dram_tensor("x_in", [ROWS, TILE_F], mybir.dt.float32, kind="ExternalInput")
    m_in = nc.dram_tensor("m_in", [128, TILE_F], mybir.dt.float32, kind="ExternalInput")
    out = nc.dram_tensor("out", [ROWS, TILE_F], mybir.dt.float32, kind="ExternalOutput")

    with tile.TileContext(nc) as tc:
        with (
            tc.tile_pool(name="maskp", bufs=1) as mp,
            tc.tile_pool(name="data", bufs=4) as dp,
        ):
            mt = mp.tile([128, TILE_F], mybir.dt.float32)
            nc.sync.dma_start(mt[:], m_in[:])
            for t in range(N_TILES):
                d = dp.tile([128, TILE_F], mybir.dt.float32)
                nc.sync.dma_start(d[:], x_in[t * 128 : (t + 1) * 128, :])
                nc.vector.tensor_mul(d[:], d[:], mt[:])
                nc.scalar.dma_start(out[t * 128 : (t + 1) * 128, :], d[:])
    nc.finalize()
    return nc


def _get_program():
    if "nc" not in _cache:
        _cache["nc"] = _build_program()
    return _cache["nc"]


def _compute_mask(cv: float) -> np.ndarray:
    """Replicates reference's mask math in numpy f32: [N, N]."""
    template = np.linspace(1.0 - MAX_SIZE, 0.0, MAX_SIZE, dtype=np.float32)
    one_d = np.clip(
        (template + np.float32(cv) * MAX_SIZE) / np.float32(RAMP_SIZE) + np.float32(1.0),
        np.float32(0.0),
        np.float32(1.0),
    ).astype(np.float32)
    one_d = one_d[-(N // 2):]  # [128]
    idx = np.arange(N)
    ring = np.minimum(
        np.minimum(idx[:, None], idx[None, :]),
        np.minimum(N - 1 - idx[:, None], N - 1 - idx[None, :]),
    )  # values in [0, 127] for N=256 — always < N//2, no center special case
    return one_d[ring]


def _run(x, current_val, **spmd_kwargs):
    x = np.ascontiguousarray(np.asarray(x), dtype=np.float32)
    cv = float(np.asarray(current_val).reshape(-1)[0])
    assert x.shape == (B, H, N, N), x.shape

    mask = _compute_mask(cv)  # [256, 256]
    # mega-mask matching the [128, 4096] tile layout (see module docstring)
    m4 = np.ascontiguousarray(np.tile(mask.reshape(16, TILE_F), (8, 1)))

    per_core = B // N_CORES
    in_maps = [
        {
            "x_in": x[c * per_core : (c + 1) * per_core].reshape(ROWS, TILE_F),
            "m_in": m4,
        }
        for c in range(N_CORES)
    ]

    nc = _get_program()
    res = run_bass_kernel_spmd(nc, in_maps, list(range(N_CORES)), **spmd_kwargs)
    out = np.concatenate(
        [r["out"].reshape(per_core, H, N, N) for r in res.results], axis=0
    )
    return out, res


def kernel(x, current_val):
    return _run(x, current_val)[0]


if __name__ == "__main__":
    xs = np.random.randn(B, H, N, N).astype(np.float32)
    cv = np.array([0.1], dtype=np.float32)
    o = kernel(x=xs, current_val=cv)
    expected = xs * _compute_mask(0.1)
    print("self-check max abs diff:", np.abs(o - expected).max())
